# revision 1
# baseline (speedup 1.0000x reference)
"""GAT (2-layer, PPI config) on 8 trn2 NeuronCores.

Math: per layer, att = softmax_row(mask(leaky_relu(f_src[d] + f_dst[s]))).
With x = f_src + f_dst and alpha = 0.2:
    exp(lrelu(x)) = max(exp(x), exp(0.2 x)) = exp(x) * max(1, exp(-0.8 x))
                  = exp(f_src[d]) * exp(f_dst[s]) * G[s, d],
    G = max(1, R[d] * r[s]),  R = exp(-0.8 f_src), r = exp(-0.8 f_dst).
Softmax-normalizing cancels exp(f_src[d]); exp(f_dst[s]) folds into the
aggregation operand (Wh' = exp(f_dst) * Wh, plus a ones->exp(f_dst) column
that accumulates the softmax denominator).  Per (s, d) element the device
computes only G (tensor_scalar, bf16 4x mode) and G*adjT (tensor_tensor,
bf16 2x mode), then a bf16 matmul.  Normalization/elu happen on host.

Sharding (8 cores), sized so each PSUM accumulator set fits (heads*D <= 4096
fp32 words per partition) while DVE ops stay wide (per-op overhead ~200ns):
  L1 (4 heads): 4 destination ranges x 2 head-pairs, D=2048.
  L2 (1 head):  4 destination ranges x 2 source halves, D=2048; the host
                adds the two partial accumulator sets.
Two launches; the tiny inter-layer tensors are re-prepped on host.
"""

import os
import sys

sys.path.insert(0, "/opt/trn_rl_repo")

import numpy as np
import ml_dtypes

import concourse.bass as bass
import concourse.tile as tile
from concourse import bacc, mybir
from concourse.bass_utils import run_bass_kernel_spmd

BF16 = mybir.dt.bfloat16
F32 = mybir.dt.float32
NPBF16 = ml_dtypes.bfloat16

N = 8192
NFEAT = 256
NHID = 64
NHEADS = 4
NCLASS = 121
ALPHA = 0.2
N_CORES = 8
P = 128

_NC_CACHE = {}
_LAST_EXEC_NS = []


def build_att_kernel(n_heads, dh, n_stiles, D, warmup=20,
                     act10=(10, 4)):
    """One attention-layer shard, per-core program.

    Inputs (per core):
      adjt [n_stiles*128, D]    bf16  adjacency slice, rows = source nodes,
                                      cols = this core's destination range
      whp  [128, n_stiles*M]    bf16  pre-tiled stationary operand: per
                                      s-tile, per head, dh cols of
                                      exp(f_dst)*Wh then 1 col exp(f_dst)
      rsc  [128, n_stiles*H]    f32   pre-tiled r = exp(-0.8 f_dst)
      rbc  [128, H*D]           bf16  R = exp(-0.8 f_src[d_range]), bcast
    Output:
      out [H*(dh+1), D] f32  raw accumulators: per head dh numerator rows
                             then 1 denominator row (normalize on host).
    """
    MP = 128  # stationary cols padded to 128 so FWL (fast weight load) engages
    M = n_heads * MP
    assert dh + 1 <= MP and n_heads * D * 4 <= 16384
    nc = bacc.Bacc("TRN2", target_bir_lowering=False, debug=False,
                   num_devices=N_CORES)
    adjt_d = nc.dram_tensor("adjt", [n_stiles * P, D], BF16,
                            kind="ExternalInput")
    whp_d = nc.dram_tensor("whp", [P, n_stiles * M], BF16,
                           kind="ExternalInput")
    rsc_d = nc.dram_tensor("rsc", [P, n_stiles * n_heads], F32,
                           kind="ExternalInput")
    rbc_d = nc.dram_tensor("rbc", [P, n_heads * D], BF16,
                           kind="ExternalInput")
    rbl_d = nc.dram_tensor("rbl", [P, n_heads * D], F32,
                           kind="ExternalInput")
    rsl_d = nc.dram_tensor("rsl", [P, n_stiles * n_heads], F32,
                           kind="ExternalInput")
    out_d = nc.dram_tensor("out", [n_heads * (dh + 1), D], F32,
                           kind="ExternalOutput")

    with tile.TileContext(nc) as tc:
        with (
            tc.tile_pool(name="const", bufs=1) as cpool,
            tc.tile_pool(name="adj", bufs=6) as apool,
            tc.tile_pool(name="g", bufs=5) as gpool,
            tc.tile_pool(name="ga", bufs=5) as gapool,
            tc.tile_pool(name="att", bufs=8) as attpool,
            tc.tile_pool(name="fin", bufs=2) as fpool,
            tc.tile_pool(name="tmp", bufs=3) as tpool,
            tc.tile_pool(name="acc", bufs=n_heads,
                         space=bass.MemorySpace.PSUM) as pspool,
        ):
            # First adjacency tiles ahead of the bulky const loads so the
            # vector engine's first mask op isn't queued behind them.
            adj_pre = []
            for st in range(min(6, n_stiles)):
                adjp = apool.tile([P, D], BF16, name=f"adjp{st}", tag="adj")
                nc.sync.dma_start(adjp[:], adjt_d[st * P:(st + 1) * P, :])
                adj_pre.append(adjp)
            rsc = cpool.tile([P, n_stiles * n_heads], F32)
            nc.sync.dma_start(rsc[:], rsc_d[:])
            rsl = cpool.tile([P, n_stiles * n_heads], F32)
            nc.sync.dma_start(rsl[:], rsl_d[:])
            rbc = cpool.tile([P, n_heads * D], BF16)
            nc.sync.dma_start(rbc[:], rbc_d[:])
            rbl = cpool.tile([P, n_heads * D], F32)
            nc.sync.dma_start(rbl[:], rbl_d[:])
            whp = cpool.tile([P, n_stiles * M], BF16)
            nc.sync.dma_start(whp[:], whp_d[:])

            accs = [pspool.tile([MP, D], F32, tag="acc", name=f"acc{i}")
                    for i in range(n_heads)]

            if warmup:
                # Dense matmul burst so the PE HAM un-throttles to 2.4 GHz
                # before the steady-state (sparser) matmul stream begins.
                wN = min(512, D)
                dmy = cpool.tile([P, wN], BF16)
                nc.vector.memset(dmy[:], 0.0)
                for w in range(warmup):
                    nc.tensor.matmul(accs[0][:, 0:wN],
                                     dmy[:, 0:wN][:, 0:MP] if wN >= MP
                                     else dmy[:, 0:wN],
                                     dmy[:, 0:wN], start=True, stop=True)

            for st in range(n_stiles):
                if st < len(adj_pre):
                    adj = adj_pre[st]
                else:
                    adj = apool.tile([P, D], BF16, tag="adj")
                    nc.sync.dma_start(adj[:], adjt_d[st * P:(st + 1) * P, :])
                head_order = sorted(range(n_heads),
                                    key=lambda hh: st % 10 < act10[hh])
                for h in head_order:
                    # adjt holds adj*1e30, so masking is a min() with the
                    # clamped gate.  Per head, either the (otherwise idle)
                    # ScalarE computes G = Exp(Relu(-0.8x)) in two LUT ops,
                    # or DVE computes G = max(R*r, 1) in one 4x-mode
                    # tensor_scalar; DVE then min-masks (2x tensor_tensor).
                    if st % 10 < act10[h] and st >= 4:
                        g = gapool.tile([P, D], BF16, name="g_act")
                        t = tpool.tile([P, D], F32)
                        nc.scalar.activation(
                            t[:], rbl[:, h * D:(h + 1) * D],
                            mybir.ActivationFunctionType.Relu,
                            bias=rsl[:, st * n_heads + h:
                                     st * n_heads + h + 1])
                        nc.scalar.activation(
                            g[:], t[:], mybir.ActivationFunctionType.Exp)
                    else:
                        g = gpool.tile([P, D], BF16)
                        nc.vector.tensor_scalar(
                            g[:], rbc[:, h * D:(h + 1) * D],
                            rsc[:, st * n_heads + h:st * n_heads + h + 1],
                            1.0, mybir.AluOpType.mult, mybir.AluOpType.max)
                    att = attpool.tile([P, D], BF16)
                    nc.vector.tensor_tensor(att[:], g[:], adj[:],
                                            mybir.AluOpType.min)
                    lhs = whp[:, st * M + h * MP:st * M + (h + 1) * MP]
                    for j0 in range(0, D, 512):
                        j1 = min(j0 + 512, D)
                        nc.tensor.matmul(
                            accs[h][:, j0:j1], lhs, att[:, j0:j1],
                            start=(st == 0), stop=(st == n_stiles - 1))

            # Raw accumulators out; host normalizes (and applies elu).
            # Output DMA split into 32-row chunks to spread across queues.
            for h in range(n_heads):
                stg = fpool.tile([dh + 1, D], F32, tag="stg")
                if h % 2 == 0:
                    nc.vector.tensor_copy(stg[:], accs[h][0:dh + 1, :])
                else:
                    nc.scalar.copy(stg[:], accs[h][0:dh + 1, :])
                for c0 in range(0, dh + 1, 32):
                    c1 = min(c0 + 32, dh + 1)
                    nc.sync.dma_start(
                        out_d[h * (dh + 1) + c0:h * (dh + 1) + c1, :],
                        stg[c0:c1, :])

    nc.compile()
    return nc


def _get_kernel(n_heads, dh, n_stiles, D, act10=(10, 4)):
    key = (n_heads, dh, n_stiles, D, act10)
    if key not in _NC_CACHE:
        _NC_CACHE[key] = build_att_kernel(n_heads, dh, n_stiles, D,
                                          act10=act10)
    return _NC_CACHE[key]


def _prep_core(Wh_heads, f_dst_heads, f_src_heads, dh, head_ids, s_range,
               d_range):
    """Host prep of whp / rsc / rbc for one core's shard."""
    s0, s1 = s_range
    n_st = (s1 - s0) // P
    H = len(head_ids)
    MP = 128
    M = H * MP
    Dc = d_range[1] - d_range[0]
    whp = np.zeros((P, n_st * M), dtype=NPBF16)
    rsc = np.empty((P, n_st * H), dtype=np.float32)
    rbc = np.empty((P, H * Dc), dtype=NPBF16)
    rbl = np.empty((P, H * Dc), dtype=np.float32)
    rsl = np.empty((P, n_st * H), dtype=np.float32)
    for i, h in enumerate(head_ids):
        fd = f_dst_heads[h][s0:s1]
        v = np.exp(fd).astype(np.float32)
        r = np.exp(-(1.0 - ALPHA) * fd).astype(np.float32)
        whv = (Wh_heads[h][s0:s1] * v[:, None]).astype(np.float32)
        aug = np.concatenate([whv, v[:, None]], axis=1)  # [s1-s0, dh+1]
        tiled = aug.reshape(n_st, P, dh + 1).astype(NPBF16)
        for st in range(n_st):
            whp[:, st * M + i * MP:st * M + i * MP + dh + 1] = tiled[st]
        rsc[:, np.arange(n_st) * H + i] = r.reshape(n_st, P).T
        R = np.exp(-(1.0 - ALPHA)
                   * f_src_heads[h][d_range[0]:d_range[1]]).astype(NPBF16)
        rbc[:, i * Dc:(i + 1) * Dc] = R[None, :]
        rsl[:, np.arange(n_st) * H + i] = \
            (-(1.0 - ALPHA) * fd).astype(np.float32).reshape(n_st, P).T
        rbl[:, i * Dc:(i + 1) * Dc] = (-(1.0 - ALPHA)
            * f_src_heads[h][d_range[0]:d_range[1]]).astype(np.float32)[None, :]
    return whp, rsc, rbc, rbl, rsl


def _launch(nc, in_maps):
    trace = bool(os.environ.get("GAT_TRACE"))
    res = run_bass_kernel_spmd(nc, in_maps, list(range(N_CORES)), trace=trace)
    if trace:
        _LAST_EXEC_NS.append(res.exec_time_ns)
    return [res.results[c]["out"] for c in range(N_CORES)]


def kernel(x, adj, Ws, a_heads, W_out, a_out):
    _LAST_EXEC_NS.clear()
    x = np.asarray(x, dtype=np.float32)
    adj = np.asarray(adj, dtype=np.float32)
    Ws = np.asarray(Ws, dtype=np.float32)
    a_heads = np.asarray(a_heads, dtype=np.float32)
    W_out = np.asarray(W_out, dtype=np.float32)
    a_out = np.asarray(a_out, dtype=np.float32)

    # ---- Layer 1: 4 d-ranges (D=2048) x 2 head-pairs ----
    D1 = N // 4
    Wh = [x @ Ws[h] for h in range(NHEADS)]
    f_src = [Wh[h] @ a_heads[h][:NHID] for h in range(NHEADS)]
    f_dst = [Wh[h] @ a_heads[h][NHID:] for h in range(NHEADS)]
    nc1 = _get_kernel(2, NHID, N // P, D1, act10=(8, 0))
    adjt_q = [np.ascontiguousarray(
        (adj[q * D1:(q + 1) * D1, :].T * 1e30).astype(NPBF16))
        for q in range(4)]
    in_maps = []
    for c in range(N_CORES):
        hg, q = c // 4, c % 4
        whp, rsc, rbc, rbl, rsl = _prep_core(Wh, f_dst, f_src, NHID,
                                             [2 * hg, 2 * hg + 1], (0, N),
                                             (q * D1, (q + 1) * D1))
        in_maps.append({"adjt": adjt_q[q], "whp": whp, "rsc": rsc,
                        "rbc": rbc, "rbl": rbl, "rsl": rsl})
    outs = _launch(nc1, in_maps)
    h_cat = np.empty((N, NHEADS * NHID), dtype=np.float32)
    for c in range(N_CORES):
        hg, q = c // 4, c % 4
        o = outs[c]  # [2*(NHID+1), D1]
        for i in range(2):
            h = 2 * hg + i
            num = o[i * (NHID + 1):i * (NHID + 1) + NHID, :]
            den = o[i * (NHID + 1) + NHID, :]
            ht = (num / den[None, :]).T  # [D1, NHID]
            h_cat[q * D1:(q + 1) * D1, h * NHID:(h + 1) * NHID] = \
                np.where(ht > 0, ht, np.expm1(np.minimum(ht, 0)))

    # ---- Layer 2: 4 d-ranges (D=2048) x 2 source halves ----
    Wh2 = h_cat @ W_out
    f_src2 = Wh2 @ a_out[:NCLASS]
    f_dst2 = Wh2 @ a_out[NCLASS:]
    nc2 = _get_kernel(1, NCLASS, N // 2 // P, D1, act10=(4,))
    in_maps = []
    for c in range(N_CORES):
        sh, q = c // 4, c % 4
        s_range = (sh * (N // 2), (sh + 1) * (N // 2))
        whp, rsc, rbc, rbl, rsl = _prep_core([Wh2], [f_dst2], [f_src2],
                                             NCLASS, [0], s_range,
                                             (q * D1, (q + 1) * D1))
        adjt = np.ascontiguousarray(
            (adj[q * D1:(q + 1) * D1, s_range[0]:s_range[1]].T
             * 1e30).astype(NPBF16))
        in_maps.append({"adjt": adjt, "whp": whp, "rsc": rsc, "rbc": rbc,
                        "rbl": rbl, "rsl": rsl})
    outs2 = _launch(nc2, in_maps)
    out = np.empty((N, NCLASS), dtype=np.float32)
    for q in range(4):
        o = outs2[q] + outs2[q + 4]  # add the two source-half partials
        out[q * D1:(q + 1) * D1, :] = (o[:NCLASS, :]
                                       / o[NCLASS, :][None, :]).T
    return out

